# revision 1
# baseline (speedup 1.0000x reference)
"""Trainium2 Bass kernel for the unrolled-GRU + FC-head problem.

Math (per example b):
    gi[t] = x[t] @ w_ih.T + b_ih                       # [T, 3H]
    gh    = h  @ w_hh.T + b_hh                         # per step
    r = sig(gi_r + gh_r); z = sig(gi_z + gh_z)
    n = tanh(gi_n + r * gh_n)
    h = (1 - z) * n + z * h                            # T sequential steps
    out = relu(h @ w_fc1.T + b_fc1) @ w_fc2.T + b_fc2  # [C]

Sharding: data-parallel over batch. B=512 over 8 cores -> B_local=64.

Per-core design (matmul operands are float32r -- full-rate 1 col/cycle on
the PE for N>=256, measured ~7e-5 relative error, far better than tf32):
  - batch is the matmul *stationary* operand: lhsT = h^T chunk [K=128, M=64],
    weights stream as the moving operand (rhs = w^T [128, N<=512]).
  - PSUM G    [64,1536]: b_ih+b_hh (r,z) + x-proj + h-proj accumulated
  - PSUM Gin  [64, 768]: b_ih(n) + x-proj(n)
  - PSUM Ghn  [64, 768]: b_hh(n) + h-proj(n)
  - biases folded into PSUM via rank-1 matmuls (ones[1,64] stationary).
  - h^T for the next step produced by 6 PE transposes + one DVE copy
    (the copy also performs the f32 -> f32r rounding the verifier needs).
"""

import os
import sys

import numpy as np

if "/opt/trn_rl_repo" not in sys.path:
    sys.path.insert(0, "/opt/trn_rl_repo")

B, T, I, H, F1, C = 512, 128, 128, 768, 256, 10
NCORES = 8
BL = B // NCORES  # 64
G3 = 3 * H  # 2304
H2 = 2 * H  # 1536
KC = H // 128  # 6 k-chunks of the hidden dim

# v2 experiment knobs
COLTILE = os.environ.get("GRU_COLTILE", "0") == "1"
TAILOPT = os.environ.get("GRU_TAILOPT", "1") == "1"
# dummy matmuls per step to keep the PE HAM clock-gate warm during the
# elementwise tail (each ~160ns of PE work into the scratch transpose bank)
FILLER = int(os.environ.get("GRU_FILLER", "0"))
# split the elementwise chain into H-halves so the first half's h/hT feeds
# the PE while the second half is still being computed
HALVES = int(os.environ.get("GRU_HALVES", "1"))
# timing-bisect knobs (produce WRONG results; for attribution only)
SKIP_ELEM = os.environ.get("GRU_SKIP_ELEM", "0") == "1"
SKIP_H = os.environ.get("GRU_SKIP_H", "0") == "1"
# bf16 h-matmuls with K-split column pairing (even k -> psum partitions
# 0:64, odd k -> 64:128, streamed concurrently; DVE folds the partials)
BF16H = os.environ.get("GRU_BF16H", "0") == "1"

_CACHE = {}


def _build_program(reps=1):
    import contextlib

    import concourse.bacc as bacc
    import concourse.mybir as mybir
    import concourse.tile as tile
    from concourse.masks import make_identity

    f32 = mybir.dt.float32
    f32r = mybir.dt.float32r
    AF = mybir.ActivationFunctionType

    nc = bacc.Bacc(
        "TRN2",
        target_bir_lowering=False,
        debug=False,
        enable_asserts=False,
        num_devices=NCORES,
    )

    def mm(out, lhsT, rhs, start, stop):
        """Matmul with batch (M=64) as stationary. With COLTILE, split the
        batch into two 32-column groups of the PE array: the two matmuls
        stream concurrently on separate XBUSes (disjoint output partitions),
        halving the weight-streaming wall time."""
        if not COLTILE:
            nc.tensor.matmul(out, lhsT, rhs, start=start, stop=stop)
            return
        hb = BL // 2
        nc.tensor.matmul(out[0:hb, :], lhsT[:, 0:hb], rhs,
                         start=start, stop=stop)
        nc.tensor.matmul(out[hb:BL, :], lhsT[:, hb:BL], rhs,
                         start=start, stop=stop)

    # ---- DRAM I/O (f32r tensors carry plain fp32 bytes from numpy) ----
    xT_d = nc.dram_tensor("xT", [128, T * BL], f32r, kind="ExternalInput")
    bf16 = mybir.dt.bfloat16
    whh_dt = bf16 if BF16H else f32r
    whhT_d = nc.dram_tensor("whhT", [128, KC * G3], whh_dt, kind="ExternalInput")
    wihT_d = nc.dram_tensor("wihT", [128, G3], f32r, kind="ExternalInput")
    brz_d = nc.dram_tensor("brz", [1, H2], f32r, kind="ExternalInput")
    bin_d = nc.dram_tensor("bin", [1, H], f32r, kind="ExternalInput")
    bhn_d = nc.dram_tensor("bhn", [1, H], f32r, kind="ExternalInput")
    ones_d = nc.dram_tensor("ones", [1, BL], f32r, kind="ExternalInput")
    wfc1T_d = nc.dram_tensor("wfc1T", [128, KC * F1], f32r, kind="ExternalInput")
    bfc1_d = nc.dram_tensor("bfc1", [1, F1], f32r, kind="ExternalInput")
    wfc2T_d = nc.dram_tensor("wfc2T", [128, 2 * C], f32r, kind="ExternalInput")
    bfc2_d = nc.dram_tensor("bfc2", [1, C], f32r, kind="ExternalInput")
    out_d = nc.dram_tensor("logits", [BL, C], f32, kind="ExternalOutput")

    with tile.TileContext(nc) as tc:
        with (
            tc.tile_pool(name="const", bufs=1) as const,
            tc.tile_pool(name="state", bufs=2) as state,
            tc.tile_pool(name="work", bufs=2) as work,
            tc.tile_pool(name="gpsum", bufs=1, space="PSUM") as gpsum,
            tc.tile_pool(name="tpsum", bufs=1, space="PSUM") as tpsum,
        ):
            # ---- constants: DMA everything in once ----
            def load(name, shape, dram):
                t_ = const.tile(shape, f32r, tag=name)
                nc.sync.dma_start(out=t_[:], in_=dram.ap())
                return t_

            xT = load("xT", [128, T * BL], xT_d)
            wihT = load("wihT", [128, G3], wihT_d)
            brz = load("brz", [1, H2], brz_d)
            bin_ = load("bin", [1, H], bin_d)
            bhn = load("bhn", [1, H], bhn_d)
            ones = load("ones", [1, BL], ones_d)
            whhT = const.tile([128, KC * G3], whh_dt, tag="whhT")
            nc.sync.dma_start(out=whhT[:], in_=whhT_d.ap())
            wfc1T = load("wfc1T", [128, KC * F1], wfc1T_d)
            bfc1 = load("bfc1", [1, F1], bfc1_d)
            wfc2T = load("wfc2T", [128, 2 * C], wfc2T_d)
            bfc2 = load("bfc2", [1, C], bfc2_d)

            ident = const.tile([BL, BL], f32, tag="ident")
            make_identity(nc, ident[:])

            h_prev = None  # SBUF [64, 768] fp32
            hT = None  # SBUF [128, KC*64] f32r (transposed h)

            def transpose_h(h_sb, hT_dt=None):
                # 6 PE transposes; PSUM->SBUF copies per half so the first
                # half of hT is available while the rest transposes.
                if hT_dt is None:
                    hT_dt = bf16 if BF16H else f32r
                Tps = tpsum.tile([128, KC * BL], f32, tag="T")
                hT_new = state.tile([128, KC * BL], hT_dt, tag="hT")
                half = KC * BL // 2  # 192
                for k in range(KC):
                    nc.tensor.transpose(
                        Tps[:, k * BL : (k + 1) * BL],
                        h_sb[:, k * 128 : (k + 1) * 128],
                        ident[:],
                    )
                    if k == KC // 2 - 1:
                        nc.vector.tensor_copy(hT_new[:, 0:half], Tps[:, 0:half])
                nc.vector.tensor_copy(hT_new[:, half:], Tps[:, half:])
                return hT_new

            def emit_body():
                emit_recurrence()
                emit_fc_head()

            def emit_recurrence():
                nonlocal h_prev, hT
                for t in range(T):
                    emit_step(t)

            def emit_step(t):
                nonlocal h_prev, hT
                gp = 128 if BF16H else BL
                G = gpsum.tile([gp, H2], f32, tag="G")
                Gin = gpsum.tile([BL, H], f32, tag="Gin")
                Ghn = gpsum.tile([gp, H], f32, tag="Ghn")
                G0, Ghn0 = G[0:BL, :], Ghn[0:BL, :]
                xt = xT[:, t * BL : (t + 1) * BL]

                # -- PE: bias init (start=True claims each bank) --
                for c0 in range(0, H2, 512):
                    mm(G0[:, c0 : c0 + 512], ones[:], brz[:, c0 : c0 + 512],
                       start=True, stop=False)
                for c0, c1 in ((0, 512), (512, 768)):
                    mm(Gin[:, c0:c1], ones[:], bin_[:, c0:c1],
                       start=True, stop=False)
                    mm(Ghn0[:, c0:c1], ones[:], bhn[:, c0:c1],
                       start=True, stop=(t == 0 or SKIP_H))

                # -- PE: x projection --
                for c0 in range(0, H2, 512):
                    mm(G0[:, c0 : c0 + 512], xt, wihT[:, c0 : c0 + 512],
                       start=False, stop=(t == 0 or SKIP_H))
                for c0, c1 in ((0, 512), (512, 768)):
                    mm(Gin[:, c0:c1], xt, wihT[:, H2 + c0 : H2 + c1],
                       start=False, stop=True)

                if FILLER and t > 0:
                    # dummy PE work into the scratch transpose bank; keeps
                    # the HAM activity window busy while ACT/DVE finish the
                    # previous step's gates. Results are never read.
                    Fps = tpsum.tile([128, KC * BL], f32, tag="T")
                    for _ in range(FILLER):
                        nc.tensor.matmul(Fps[0:64, :], xt,
                                         whhT[:, 0 : KC * BL],
                                         start=True, stop=True)

                if t > 0 and not SKIP_H:
                    # -- PE: transpose h_{t-1} -> hT, then h projection --
                    hT = transpose_h(h_prev)
                    for k in range(KC):
                        hk = hT[:, k * BL : (k + 1) * BL]
                        wk = k * G3
                        if BF16H:
                            odd = k % 2 == 1
                            Gt = G[64:128, :] if odd else G0
                            Ghnt = Ghn[64:128, :] if odd else Ghn0
                            st = odd and k == 1  # odd chain opens at k=1
                            last = k >= KC - 2
                        else:
                            Gt, Ghnt, st = G0, Ghn0, False
                            last = k == KC - 1

                        def g_mms():
                            for c0 in range(0, H2, 512):
                                mm(Gt[:, c0 : c0 + 512], hk,
                                   whhT[:, wk + c0 : wk + c0 + 512],
                                   start=st, stop=last)

                        def hn_mms():
                            for c0, c1 in ((0, 512), (512, 768)):
                                mm(Ghnt[:, c0:c1], hk,
                                   whhT[:, wk + H2 + c0 : wk + H2 + c1],
                                   start=st, stop=last)

                        # last k-group: finish Ghn and the r-region chunks
                        # first so the r-sigmoid / tn chain starts earlier
                        # (the z-region chunk [1024:1536] stops last).
                        if last:
                            hn_mms()
                            g_mms()
                        else:
                            g_mms()
                            hn_mms()

                # -- ACT/DVE: gates + state update --
                h_new = state.tile([BL, H], f32, tag="h")
                if SKIP_ELEM:
                    nc.vector.tensor_copy(h_new[:], G[0:BL, 0:H])
                    h_prev = h_new
                    return
                elif TAILOPT:
                    if BF16H and t > 0:
                        gf = work.tile([BL, H2], f32, tag="gf")
                        hnf = work.tile([BL, H], f32, tag="hnf")
                        nc.vector.tensor_add(gf[:, 0:H], G[0:BL, 0:H],
                                             G[64:128, 0:H])
                        nc.vector.tensor_add(gf[:, H:H2], G[0:BL, H:H2],
                                             G[64:128, H:H2])
                        nc.vector.tensor_add(hnf[:], Ghn[0:BL, :],
                                             Ghn[64:128, :])
                        Gv, Ghnv = gf, hnf
                    else:
                        Gv, Ghnv = G0, Ghn0
                    # per H-half chains: r -> tn -> tn2 -> tanh -> w1 -> h.
                    # z / u=z*h / vm=z-1 run in the shadow; ACT only does
                    # sigmoids+tanh (vm on DVE):  h = u - vm*n.
                    nh = HALVES
                    hw_ = H // nh
                    r_s = work.tile([BL, H], f32, tag="r")
                    z_s = work.tile([BL, H], f32, tag="z")
                    tn = work.tile([BL, H], f32, tag="tn")
                    tn2 = work.tile([BL, H], f32, tag="tn2")
                    n_t = work.tile([BL, H], f32, tag="n")
                    u = work.tile([BL, H], f32, tag="u")
                    vm = work.tile([BL, H], f32, tag="vm")
                    w1 = work.tile([BL, H], f32, tag="w1")
                    sl = [slice(i * hw_, (i + 1) * hw_) for i in range(nh)]
                    for s in sl:
                        nc.scalar.activation(r_s[:, s], Gv[:, s.start : s.stop],
                                             AF.Sigmoid)
                    for s in sl:
                        nc.vector.tensor_mul(tn[:, s], r_s[:, s], Ghnv[:, s])
                        nc.vector.tensor_add(tn2[:, s], tn[:, s], Gin[:, s])
                    for s in sl:
                        nc.scalar.activation(
                            z_s[:, s], Gv[:, H + s.start : H + s.stop],
                            AF.Sigmoid)
                    for s in sl:
                        nc.scalar.activation(n_t[:, s], tn2[:, s], AF.Tanh)
                    for s in sl:
                        if t > 0:
                            nc.vector.tensor_mul(u[:, s], z_s[:, s],
                                                 h_prev[:, s])  # z*h
                        nc.vector.tensor_scalar_sub(vm[:, s], z_s[:, s],
                                                    1.0)  # z-1
                    for s in sl:
                        nc.vector.tensor_mul(w1[:, s], vm[:, s],
                                             n_t[:, s])  # (z-1)*n
                        if t > 0:
                            nc.vector.tensor_sub(h_new[:, s], u[:, s],
                                                 w1[:, s])  # z*h+(1-z)*n
                        else:
                            nc.vector.tensor_scalar_mul(h_new[:, s],
                                                        w1[:, s], -1.0)
                else:
                    rz = work.tile([BL, H2], f32, tag="rz")
                    nc.scalar.activation(rz[:], G0[:], AF.Sigmoid)
                    tn = work.tile([BL, H], f32, tag="tn")
                    nc.vector.tensor_mul(tn[:], rz[:, 0:H], Ghn0[:])
                    tn2 = work.tile([BL, H], f32, tag="tn2")
                    nc.vector.tensor_add(tn2[:], tn[:], Gin[:])
                    n_t = work.tile([BL, H], f32, tag="n")
                    nc.scalar.activation(n_t[:], tn2[:], AF.Tanh)
                    if t == 0:
                        v = work.tile([BL, H], f32, tag="d")
                        nc.scalar.activation(v[:], rz[:, H:H2], AF.Copy,
                                             bias=1.0, scale=-1.0)  # 1 - z
                        nc.vector.tensor_mul(h_new[:], v[:], n_t[:])
                    else:
                        d = work.tile([BL, H], f32, tag="d")
                        nc.vector.tensor_sub(d[:], h_prev[:], n_t[:])
                        m = work.tile([BL, H], f32, tag="m")
                        nc.vector.tensor_mul(m[:], rz[:, H:H2], d[:])
                        nc.vector.tensor_add(h_new[:], n_t[:], m[:])
                h_prev = h_new

            def emit_fc_head():
                nonlocal h_prev, hT
                hT = transpose_h(h_prev, hT_dt=f32r)
                fc1 = gpsum.tile([BL, F1], f32, tag="G")
                mm(fc1[:], ones[:], bfc1[:], start=True, stop=False)
                for k in range(KC):
                    mm(fc1[:], hT[:, k * BL : (k + 1) * BL],
                       wfc1T[:, k * F1 : (k + 1) * F1],
                       start=False, stop=(k == KC - 1))
                o1 = work.tile([BL, F1], f32, tag="o1")
                nc.scalar.activation(o1[:], fc1[:], AF.Relu)

                T2 = tpsum.tile([128, 2 * BL], f32, tag="T")
                nc.tensor.transpose(T2[:, 0:BL], o1[:, 0:128], ident[:])
                nc.tensor.transpose(T2[:, BL : 2 * BL], o1[:, 128:256], ident[:])
                o1T = work.tile([128, 2 * BL], f32r, tag="o1T")
                nc.vector.tensor_copy(o1T[:], T2[:])

                fc2 = gpsum.tile([BL, C], f32, tag="Gin")
                mm(fc2[:], ones[:], bfc2[:], start=True, stop=False)
                mm(fc2[:], o1T[:, 0:BL], wfc2T[:, 0:C], start=False, stop=False)
                mm(fc2[:], o1T[:, BL : 2 * BL], wfc2T[:, C : 2 * C],
                   start=False, stop=True)
                lo = work.tile([BL, C], f32, tag="lo")
                nc.vector.tensor_copy(lo[:], fc2[:])
                nc.sync.dma_start(out=out_d.ap(), in_=lo[:])

            # bench mode: repeat the whole computation in a HW loop so the
            # per-iteration time can be extracted from noisy wall-clock.
            if reps > 1:
                with tc.For_i(0, reps, 1):
                    emit_body()
            else:
                emit_body()

    nc.compile()
    return nc


def _prep_shared(w_ih, w_hh, b_ih, b_hh, w_fc1, b_fc1, w_fc2, b_fc2):
    f = np.float32

    def kmajor(wT, kc, n):  # [kc*128, n] -> [128, kc*n]
        return np.ascontiguousarray(
            wT.reshape(kc, 128, n).transpose(1, 0, 2).reshape(128, kc * n)
        ).astype(f, copy=False)

    whhT = kmajor(np.ascontiguousarray(w_hh.T), KC, G3)
    wihT = np.ascontiguousarray(w_ih.T).astype(f, copy=False)
    b_sum = (b_ih + b_hh).astype(f)
    if BF16H:
        import ml_dtypes
        whhT = whhT.astype(ml_dtypes.bfloat16)
    shared = {
        "whhT": whhT,
        "wihT": wihT,
        "brz": np.ascontiguousarray(b_sum[None, :H2]),
        "bin": np.ascontiguousarray(b_ih.astype(f)[None, H2:G3]),
        "bhn": np.ascontiguousarray(b_hh.astype(f)[None, H2:G3]),
        "ones": np.ones((1, BL), f),
        "wfc1T": kmajor(np.ascontiguousarray(w_fc1.T), KC, F1),
        "bfc1": np.ascontiguousarray(b_fc1.astype(f)[None, :]),
        "wfc2T": kmajor(np.ascontiguousarray(w_fc2.T), 2, C),
        "bfc2": np.ascontiguousarray(b_fc2.astype(f)[None, :]),
    }
    return shared


def _prep_in_maps(inputs):
    x = np.asarray(inputs["x"], dtype=np.float32)
    shared = _prep_shared(
        *(np.asarray(inputs[k], dtype=np.float32)
          for k in ("w_ih", "w_hh", "b_ih", "b_hh", "w_fc1", "b_fc1",
                    "w_fc2", "b_fc2"))
    )
    in_maps = []
    for c in range(NCORES):
        xs = x[c * BL : (c + 1) * BL]  # [64, T, I]
        xT = np.ascontiguousarray(xs.transpose(2, 1, 0).reshape(128, T * BL))
        in_maps.append({**shared, "xT": xT})
    return in_maps


def _execute(in_maps, reps=1):
    from concourse.bass_utils import run_bass_kernel_spmd

    key = ("nc", reps)
    if key not in _CACHE:
        _CACHE[key] = _build_program(reps=reps)
    nc = _CACHE[key]
    res = run_bass_kernel_spmd(nc, in_maps, core_ids=list(range(NCORES)))
    out = np.concatenate([res.results[c]["logits"] for c in range(NCORES)], axis=0)
    return out.astype(np.float32), res


def _run(inputs, trace=False, trace_kwargs=None):
    return _execute(_prep_in_maps(inputs))


def kernel(**inputs):
    out, _ = _execute(_prep_in_maps(inputs))
    return out



# revision 3
# speedup vs baseline: 1.3864x; 1.3864x over previous
"""Trainium2 Bass kernel for the unrolled-GRU + FC-head problem (v2).

Math (per example b):
    gi[t] = x[t] @ w_ih.T + b_ih                       # [T, 3H]
    gh    = h  @ w_hh.T + b_hh                         # per step
    r = sig(gi_r + gh_r); z = sig(gi_z + gh_z)
    n = tanh(gi_n + r * gh_n)
    h = (1 - z) * n + z * h                            # T sequential steps
    out = relu(h @ w_fc1.T + b_fc1) @ w_fc2.T + b_fc2  # [C]

Sharding: data-parallel over batch. B=512 over 8 cores -> B_local=64.

v2 design (vs the v1 f32r/[64,*] kernel, measured 2.475 ms):
  - STACKED layout: every [64, 768]-logical tensor is stored [128, 384]:
    partitions 0:64 hold hidden cols 0:384 (lo), partitions 64:128 hold
    cols 384:768 (hi) for the same batch rows.  All ACT/DVE tail ops use
    all 128 lanes at half the free dim -> tail time halves.
  - The hi half is produced by bf16 matmuls with tile_position=(0,64)
    (PE col groups 2,3).  bf16 everywhere on the PE (weights, x, hT);
    f32 accumulation in PSUM and an f32 tail keep the error ~1.6e-3.
  - Rank-2 bias matmuls (K=2 selector: row0->partitions 0:64, row1->
    64:128) write both halves of each bias region in one N=384 stream:
    1536 bias cols/step instead of 3072.
  - G (r,z gate sums) is double-buffered in PSUM, so step t+1's bias +
    x-projection stream on the PE while step t's tail runs on ACT/DVE.
    This both hides the work and keeps the PE HAM clock-gate warm (the
    v1 kernel sat idle > 3.4us each step and ran its matmuls at 1.2GHz).

PSUM budget: G [128,1024] x2 bufs (4 banks) + NG [128,1024] (Gin bank0,
Ghn bank1) + T [128,384] transpose scratch (1 bank) = 7 of 8 banks.
"""

import os
import sys

import numpy as np

if "/opt/trn_rl_repo" not in sys.path:
    sys.path.insert(0, "/opt/trn_rl_repo")

B, T, I, H, F1, C = 512, 128, 128, 768, 256, 10
NCORES = 8
BL = B // NCORES  # 64
G3 = 3 * H  # 2304
KC = H // 128  # 6 k-chunks of the hidden dim
HW = H // 2  # 384 stacked half-width

_CACHE = {}


def _build_program(reps=1):
    import concourse.bacc as bacc
    import concourse.mybir as mybir
    import concourse.tile as tile
    from concourse.masks import make_identity

    f32 = mybir.dt.float32
    bf16 = mybir.dt.bfloat16
    AF = mybir.ActivationFunctionType

    nc = bacc.Bacc(
        "TRN2",
        target_bir_lowering=False,
        debug=False,
        enable_asserts=False,
        num_devices=NCORES,
    )

    # ---- DRAM I/O ----
    xT_d = nc.dram_tensor("xT", [128, T * BL], bf16, kind="ExternalInput")
    whh_d = nc.dram_tensor("whhTs", [128, KC * G3], bf16, kind="ExternalInput")
    wih_d = nc.dram_tensor("wihTs", [128, G3], bf16, kind="ExternalInput")
    bias2_d = nc.dram_tensor("bias2", [2, 4 * HW], bf16, kind="ExternalInput")
    sel2_d = nc.dram_tensor("sel2", [2, 128], bf16, kind="ExternalInput")
    ones_d = nc.dram_tensor("ones", [1, BL], bf16, kind="ExternalInput")
    wfc1_d = nc.dram_tensor("wfc1Ts", [128, KC * F1], bf16, kind="ExternalInput")
    bfc1_d = nc.dram_tensor("bfc1", [1, F1], bf16, kind="ExternalInput")
    wfc2_d = nc.dram_tensor("wfc2Ts", [128, 2 * C], bf16, kind="ExternalInput")
    bfc2_d = nc.dram_tensor("bfc2", [1, C], bf16, kind="ExternalInput")
    out_d = nc.dram_tensor("logits", [BL, C], f32, kind="ExternalOutput")

    LO, HI = (0, 0), (0, 64)  # matmul tile_position for lo/hi output halves

    with tile.TileContext(nc) as tc:
        with (
            tc.tile_pool(name="const", bufs=1) as const,
            tc.tile_pool(name="state", bufs=2) as state,
            tc.tile_pool(name="work", bufs=2) as work,
            tc.tile_pool(name="gpsum", bufs=2, space="PSUM") as gpsum,
            tc.tile_pool(name="npsum", bufs=1, space="PSUM") as npsum,
            tc.tile_pool(name="tpsum", bufs=1, space="PSUM") as tpsum,
        ):
            def load(name, shape, dram, dt=bf16):
                t_ = const.tile(shape, dt, tag=name)
                nc.sync.dma_start(out=t_[:], in_=dram.ap())
                return t_

            xT = load("xT", [128, T * BL], xT_d)
            whhTs = load("whhTs", [128, KC * G3], whh_d)
            wihTs = load("wihTs", [128, G3], wih_d)
            bias2 = load("bias2", [2, 4 * HW], bias2_d)
            sel2 = load("sel2", [2, 128], sel2_d)
            ones = load("ones", [1, BL], ones_d)
            wfc1Ts = load("wfc1Ts", [128, KC * F1], wfc1_d)
            bfc1 = load("bfc1", [1, F1], bfc1_d)
            wfc2Ts = load("wfc2Ts", [128, 2 * C], wfc2_d)
            bfc2 = load("bfc2", [1, C], bfc2_d)

            ident = const.tile([128, BL], f32, tag="ident")
            make_identity(nc, ident[0:BL, :])
            make_identity(nc, ident[BL:128, :])

            h_s = None  # SBUF [128, HW] f32, stacked hidden state

            def mm(out, lhsT, rhs, start, stop, tp):
                nc.tensor.matmul(out, lhsT, rhs, start=start, stop=stop,
                                 tile_position=tp)

            def transpose_h(src):
                """Stacked h [128, HW] -> hT bf16 [128, KC*BL] (6 chunks of
                h-dims k*128:(k+1)*128, pre-transposed for the stationary)."""
                Tps = tpsum.tile([128, KC * BL], f32, tag="T")
                hT_new = state.tile([128, KC * BL], bf16, tag="hT")
                half = KC * BL // 2  # 192
                for k in range(KC):
                    if k < KC // 2:
                        sl = src[0:BL, k * 128 : (k + 1) * 128]
                        idn, tp = ident[0:BL, :], (0, 0)
                    else:
                        c0 = (k - KC // 2) * 128
                        sl = src[BL:128, c0 : c0 + 128]
                        idn, tp = ident[BL:128, :], (64, 0)
                    nc.tensor.transpose(Tps[:, k * BL : (k + 1) * BL], sl, idn,
                                        tile_position=tp)
                    if k == KC // 2 - 1:
                        nc.vector.tensor_copy(hT_new[:, 0:half], Tps[:, 0:half])
                nc.vector.tensor_copy(hT_new[:, half:], Tps[:, half:])
                return hT_new

            def emit_step(t):
                nonlocal h_s
                G = gpsum.tile([128, 1024], f32, tag="G")
                NG = npsum.tile([128, 1024], f32, tag="NG")
                Gr, Gz = G[:, 0:HW], G[:, 512 : 512 + HW]
                Gin, Ghn = NG[:, 0:HW], NG[:, 512 : 512 + HW]

                # -- PE: G-buffer work first (double-buffered: no waits),
                # n-region (single-buffered NG) last so the PE streams the
                # free work while step t-1's tail drains its NG reads --
                xt = xT[:, t * BL : (t + 1) * BL]
                w = wihTs
                mm(Gr, sel2[:], bias2[:, 0:HW], True, False, (0, 0))
                mm(Gz, sel2[:], bias2[:, HW : 2 * HW], True, False, (0, 0))
                mm(Gr[0:BL, :], xt, w[:, 0:HW], False, t == 0, LO)
                mm(Gz[0:BL, :], xt, w[:, HW : 2 * HW], False, t == 0, LO)
                mm(Gr[BL:128, :], xt, w[:, 3 * HW : 4 * HW], False, t == 0, HI)
                mm(Gz[BL:128, :], xt, w[:, 4 * HW : 5 * HW], False, t == 0, HI)
                mm(Ghn, sel2[:], bias2[:, 3 * HW : 4 * HW], True, t == 0, (0, 0))
                mm(Gin, sel2[:], bias2[:, 2 * HW : 3 * HW], True, False, (0, 0))
                mm(Gin[0:BL, :], xt, w[:, 2 * HW : 3 * HW], False, True, LO)
                mm(Gin[BL:128, :], xt, w[:, 5 * HW : 6 * HW], False, True, HI)

                # -- PE: transpose h(t-1), then h-projection --
                if t > 0:
                    hT = transpose_h(h_s)
                    for k in range(KC):
                        hk = hT[:, k * BL : (k + 1) * BL]
                        wb = whhTs[:, k * G3 : (k + 1) * G3]
                        last = k == KC - 1
                        mm(Gr[0:BL, :], hk, wb[:, 0:HW], False, last, LO)
                        mm(Gz[0:BL, :], hk, wb[:, HW : 2 * HW], False, last, LO)
                        mm(Ghn[0:BL, :], hk, wb[:, 2 * HW : 3 * HW], False,
                           last, LO)
                        mm(Gr[BL:128, :], hk, wb[:, 3 * HW : 4 * HW], False,
                           last, HI)
                        mm(Gz[BL:128, :], hk, wb[:, 4 * HW : 5 * HW], False,
                           last, HI)
                        mm(Ghn[BL:128, :], hk, wb[:, 5 * HW : 6 * HW], False,
                           last, HI)

                # -- ACT/DVE tail (stacked [128, HW] ops) --
                r_s = work.tile([128, HW], f32, tag="r")
                z_s = work.tile([128, HW], f32, tag="z")
                tn = work.tile([128, HW], f32, tag="tn")
                tn2 = work.tile([128, HW], f32, tag="tn2")
                n_t = work.tile([128, HW], f32, tag="n")
                u = work.tile([128, HW], f32, tag="u")
                vm = work.tile([128, HW], f32, tag="vm")
                w1 = work.tile([128, HW], f32, tag="w1")
                h_new = state.tile([128, HW], f32, tag="h")

                nc.scalar.activation(r_s[:], Gr, AF.Sigmoid)
                nc.vector.tensor_mul(tn[:], r_s[:], Ghn)
                nc.scalar.activation(z_s[:], Gz, AF.Sigmoid)
                nc.vector.tensor_add(tn2[:], tn[:], Gin)
                nc.scalar.activation(n_t[:], tn2[:], AF.Tanh)
                if t > 0:
                    nc.vector.tensor_mul(u[:], z_s[:], h_s[:])  # z*h
                nc.vector.tensor_scalar_sub(vm[:], z_s[:], 1.0)  # z-1
                nc.vector.tensor_mul(w1[:], vm[:], n_t[:])  # (z-1)*n
                if t > 0:
                    nc.vector.tensor_sub(h_new[:], u[:], w1[:])
                else:
                    nc.vector.tensor_scalar_mul(h_new[:], w1[:], -1.0)
                h_s = h_new

            def emit_fc_head():
                hT = transpose_h(h_s)
                fc1 = gpsum.tile([128, 1024], f32, tag="G")
                f1 = fc1[0:BL, 0:F1]
                mm(f1, ones[:], bfc1[:], True, False, (0, 0))
                for k in range(KC):
                    mm(f1, hT[:, k * BL : (k + 1) * BL],
                       wfc1Ts[:, k * F1 : (k + 1) * F1], False, k == KC - 1,
                       LO)
                o1 = work.tile([BL, F1], f32, tag="o1")
                nc.scalar.activation(o1[:], f1, AF.Relu)

                T2 = tpsum.tile([128, KC * BL], f32, tag="T")
                nc.tensor.transpose(T2[:, 0:BL], o1[:, 0:128], ident[0:BL, :],
                                    tile_position=(0, 0))
                nc.tensor.transpose(T2[:, BL : 2 * BL], o1[:, 128:256],
                                    ident[0:BL, :], tile_position=(0, 0))
                o1T = work.tile([128, 2 * BL], bf16, tag="o1T")
                nc.vector.tensor_copy(o1T[:], T2[:, 0 : 2 * BL])

                fc2 = npsum.tile([128, 1024], f32, tag="NG")
                f2 = fc2[0:BL, 0:C]
                mm(f2, ones[:], bfc2[:], True, False, (0, 0))
                mm(f2, o1T[:, 0:BL], wfc2Ts[:, 0:C], False, False, LO)
                mm(f2, o1T[:, BL : 2 * BL], wfc2Ts[:, C : 2 * C], False, True,
                   LO)
                lo = work.tile([BL, C], f32, tag="lo")
                nc.vector.tensor_copy(lo[:], f2)
                nc.sync.dma_start(out=out_d.ap(), in_=lo[:])

            def emit_body():
                for t in range(T):
                    emit_step(t)
                emit_fc_head()

            if reps > 1:
                with tc.For_i(0, reps, 1):
                    emit_body()
            else:
                emit_body()

    nc.compile()
    return nc


def _prep_shared(w_ih, w_hh, b_ih, b_hh, w_fc1, b_fc1, w_fc2, b_fc2):
    import ml_dtypes

    bf = ml_dtypes.bfloat16
    f = np.float32

    def halves(wT):  # [K, 3H] -> [K, 6*HW]: r-lo z-lo n-lo r-hi z-hi n-hi
        out = np.empty((wT.shape[0], G3), np.float32)
        for g in range(3):  # r, z, n
            reg = wT[:, g * H : (g + 1) * H]
            out[:, g * HW : (g + 1) * HW] = reg[:, 0:HW]
            out[:, (3 + g) * HW : (4 + g) * HW] = reg[:, HW:H]
        return out

    whhT = np.ascontiguousarray(w_hh.T).astype(f)  # [H, 3H]
    whhTs = np.concatenate(
        [halves(whhT[k * 128 : (k + 1) * 128]) for k in range(KC)], axis=1
    )  # [128, KC*G3]
    wihTs = halves(np.ascontiguousarray(w_ih.T).astype(f))  # [128, G3]

    b_sum = (b_ih + b_hh).astype(f)  # for r, z regions
    bias2 = np.zeros((2, 4 * HW), np.float32)
    bias2[0, 0:HW] = b_sum[0:HW]  # r lo
    bias2[1, 0:HW] = b_sum[HW:H]  # r hi
    bias2[0, HW : 2 * HW] = b_sum[H : H + HW]  # z lo
    bias2[1, HW : 2 * HW] = b_sum[H + HW : 2 * H]  # z hi
    bias2[0, 2 * HW : 3 * HW] = b_ih[2 * H : 2 * H + HW]  # in lo
    bias2[1, 2 * HW : 3 * HW] = b_ih[2 * H + HW : 3 * H]  # in hi
    bias2[0, 3 * HW : 4 * HW] = b_hh[2 * H : 2 * H + HW]  # hn lo
    bias2[1, 3 * HW : 4 * HW] = b_hh[2 * H + HW : 3 * H]  # hn hi

    sel2 = np.zeros((2, 128), np.float32)
    sel2[0, 0:BL] = 1.0
    sel2[1, BL:128] = 1.0

    def kmajor(wT, kc, n):  # [kc*128, n] -> [128, kc*n]
        return np.ascontiguousarray(
            wT.reshape(kc, 128, n).transpose(1, 0, 2).reshape(128, kc * n)
        )

    shared = {
        "whhTs": whhTs.astype(bf),
        "wihTs": wihTs.astype(bf),
        "bias2": bias2.astype(bf),
        "sel2": sel2.astype(bf),
        "ones": np.ones((1, BL), bf),
        "wfc1Ts": kmajor(np.ascontiguousarray(w_fc1.T).astype(f), KC, F1).astype(bf),
        "bfc1": b_fc1.astype(f)[None, :].astype(bf),
        "wfc2Ts": kmajor(np.ascontiguousarray(w_fc2.T).astype(f), 2, C).astype(bf),
        "bfc2": b_fc2.astype(f)[None, :].astype(bf),
    }
    return shared


def _prep_in_maps(inputs):
    import ml_dtypes

    x = np.asarray(inputs["x"], dtype=np.float32)
    shared = _prep_shared(
        *(np.asarray(inputs[k], dtype=np.float32)
          for k in ("w_ih", "w_hh", "b_ih", "b_hh", "w_fc1", "b_fc1",
                    "w_fc2", "b_fc2"))
    )
    in_maps = []
    for c in range(NCORES):
        xs = x[c * BL : (c + 1) * BL]  # [64, T, I]
        xTc = np.ascontiguousarray(
            xs.transpose(2, 1, 0).reshape(128, T * BL)
        ).astype(ml_dtypes.bfloat16)
        in_maps.append({**shared, "xT": xTc})
    return in_maps


def _execute(in_maps, reps=1):
    from concourse.bass_utils import run_bass_kernel_spmd

    key = ("nc", reps)
    if key not in _CACHE:
        _CACHE[key] = _build_program(reps=reps)
    nc = _CACHE[key]
    res = run_bass_kernel_spmd(nc, in_maps, core_ids=list(range(NCORES)))
    out = np.concatenate([res.results[c]["logits"] for c in range(NCORES)], axis=0)
    return out.astype(np.float32), res


def kernel(**inputs):
    out, _ = _execute(_prep_in_maps(inputs))
    return out


# revision 6
# speedup vs baseline: 1.4351x; 1.0351x over previous
"""Trainium2 Bass kernel for the unrolled-GRU + FC-head problem (v2.2).

Math (per example b):
    gi[t] = x[t] @ w_ih.T + b_ih                       # [T, 3H]
    gh    = h  @ w_hh.T + b_hh                         # per step
    r = sig(gi_r + gh_r); z = sig(gi_z + gh_z)
    n = tanh(gi_n + r * gh_n)
    h = (1 - z) * n + z * h                            # T sequential steps
    out = relu(h @ w_fc1.T + b_fc1) @ w_fc2.T + b_fc2  # [C]

Sharding: data-parallel over batch. B=512 over 8 cores -> B_local=64.

Design (v1 f32r/[64,*] kernel measured 2.475 ms; v2 stacked 1.785 ms):
  - STACKED layout: every [64, 768]-logical tensor is stored [128, 384]:
    partitions 0:64 hold hidden cols 0:384 (lo), partitions 64:128 hold
    cols 384:768 (hi) for the same batch rows.  All ACT/DVE/GPS tail ops
    use all 128 lanes at half the free dim.  The hi half is produced by
    bf16 matmuls with tile_position=(0,64) (PE col groups 2,3); f32
    PSUM accumulation and an f32 tail keep rel err ~3e-3 (gate 2e-2).
  - Rank-2 bias matmuls (K=2 selector: row0 -> partitions 0:64, row1 ->
    64:128) write both halves of each bias region in one N=384 stream.
  - PSUM: G [128,1536] (Gr @0, Gz @512, Ghn @1024 - one bank each),
    double-buffered (6 banks) so step t+1's bias/x-proj stream while
    step t's tail runs (also keeps the PE HAM clock-gate warm);
    NG [128,512] holds Gin (single buffer; its bias/x are emitted after
    the tail reads of the previous Gin); T [128,512] transpose scratch.
  - 3-slice pipelined tail: h_new is produced in three 128-col stacked
    slices; each finished slice is block-transposed ([128,128] PE
    transpose = hT chunks {s, s+3} at once) and immediately feeds the
    next step's h-projection for those two k-chunks, overlapping the
    tail with PE streaming.
  - SBUF-only tail ops (u=z*h, vm=z-1, w1=vm*n, h=u-w1) run on GpSimd,
    PSUM-touching ones (tn, tn2) on DVE, sigmoids/tanh on ACT.
"""

import os
import sys

import numpy as np

if "/opt/trn_rl_repo" not in sys.path:
    sys.path.insert(0, "/opt/trn_rl_repo")

B, T, I, H, F1, C = 512, 128, 128, 768, 256, 10
NCORES = 8
BL = B // NCORES  # 64
G3 = 3 * H  # 2304
KC = H // 128  # 6 k-chunks of the hidden dim
HW = H // 2  # 384 stacked half-width
NS = 3  # tail slices
SW = HW // NS  # 128 slice width

GPS = os.environ.get("GRU_GPS", "1") == "1"

_CACHE = {}


def _build_program(reps=1):
    import concourse.bacc as bacc
    import concourse.mybir as mybir
    import concourse.tile as tile
    from concourse.masks import make_identity

    f32 = mybir.dt.float32
    bf16 = mybir.dt.bfloat16
    AF = mybir.ActivationFunctionType

    nc = bacc.Bacc(
        "TRN2",
        target_bir_lowering=False,
        debug=False,
        enable_asserts=False,
        num_devices=NCORES,
    )

    # ---- DRAM I/O ----
    xT_d = nc.dram_tensor("xT", [128, T * BL], bf16, kind="ExternalInput")
    whh_d = nc.dram_tensor("whhTs", [128, KC * G3], bf16, kind="ExternalInput")
    wih_d = nc.dram_tensor("wihTs", [128, G3], bf16, kind="ExternalInput")
    bias2_d = nc.dram_tensor("bias2", [2, 4 * HW], bf16, kind="ExternalInput")
    sel2_d = nc.dram_tensor("sel2", [2, 128], bf16, kind="ExternalInput")
    ones_d = nc.dram_tensor("ones", [1, BL], bf16, kind="ExternalInput")
    wfc1_d = nc.dram_tensor("wfc1Ts", [128, KC * F1], bf16, kind="ExternalInput")
    bfc1_d = nc.dram_tensor("bfc1", [1, F1], bf16, kind="ExternalInput")
    wfc2_d = nc.dram_tensor("wfc2Ts", [128, 2 * C], bf16, kind="ExternalInput")
    bfc2_d = nc.dram_tensor("bfc2", [1, C], bf16, kind="ExternalInput")
    out_d = nc.dram_tensor("logits", [BL, C], f32, kind="ExternalOutput")

    LO, HI = (0, 0), (0, 64)  # matmul tile_position for lo/hi output halves

    with tile.TileContext(nc) as tc:
        with (
            tc.tile_pool(name="const", bufs=1) as const,
            tc.tile_pool(name="state", bufs=2) as state,
            tc.tile_pool(name="work", bufs=2) as work,
            tc.tile_pool(name="gpsum", bufs=2, space="PSUM") as gpsum,
            tc.tile_pool(name="npsum", bufs=1, space="PSUM") as npsum,
            tc.tile_pool(name="tpsum", bufs=1, space="PSUM") as tpsum,
        ):
            def load(name, shape, dram, dt=bf16):
                t_ = const.tile(shape, dt, tag=name)
                nc.sync.dma_start(out=t_[:], in_=dram.ap())
                return t_

            xT = load("xT", [128, T * BL], xT_d)
            whhTs = load("whhTs", [128, KC * G3], whh_d)
            wihTs = load("wihTs", [128, G3], wih_d)
            bias2 = load("bias2", [2, 4 * HW], bias2_d)
            sel2 = load("sel2", [2, 128], sel2_d)
            ones = load("ones", [1, BL], ones_d)
            wfc1Ts = load("wfc1Ts", [128, KC * F1], wfc1_d)
            bfc1 = load("bfc1", [1, F1], bfc1_d)
            wfc2Ts = load("wfc2Ts", [128, 2 * C], wfc2_d)
            bfc2 = load("bfc2", [1, C], bfc2_d)

            ident = const.tile([128, 128], f32, tag="ident")
            make_identity(nc, ident[:])

            eng2 = nc.gpsimd if GPS else nc.vector

            def mm(out, lhsT, rhs, start, stop, tp):
                nc.tensor.matmul(out, lhsT, rhs, start=start, stop=stop,
                                 tile_position=tp)

            def prologue(t):
                """PE: G-buffer bias + x-proj (r, z) and Ghn bias.  G is
                double-buffered so these stream during step t-1's tail."""
                G = gpsum.tile([128, 1536], f32, tag="G")
                Gr, Gz = G[:, 0:HW], G[:, 512 : 512 + HW]
                Ghn = G[:, 1024 : 1024 + HW]
                xt = xT[:, t * BL : (t + 1) * BL]
                w = wihTs
                mm(Gr, sel2[:], bias2[:, 0:HW], True, False, (0, 0))
                mm(Gz, sel2[:], bias2[:, HW : 2 * HW], True, False, (0, 0))
                mm(Ghn, sel2[:], bias2[:, 3 * HW : 4 * HW], True, t == 0, (0, 0))
                mm(Gr[0:BL, :], xt, w[:, 0:HW], False, t == 0, LO)
                mm(Gz[0:BL, :], xt, w[:, HW : 2 * HW], False, t == 0, LO)
                mm(Gr[BL:128, :], xt, w[:, 3 * HW : 4 * HW], False, t == 0, HI)
                mm(Gz[BL:128, :], xt, w[:, 4 * HW : 5 * HW], False, t == 0, HI)
                return G

            def gin_part(t, NG):
                """PE: Gin bias + x-proj (single-buffered NG: emitted after
                the previous tail's Gin reads)."""
                Gin = NG[:, 0:HW]
                xt = xT[:, t * BL : (t + 1) * BL]
                w = wihTs
                mm(Gin, sel2[:], bias2[:, 2 * HW : 3 * HW], True, False, (0, 0))
                mm(Gin[0:BL, :], xt, w[:, 2 * HW : 3 * HW], False, True, LO)
                mm(Gin[BL:128, :], xt, w[:, 5 * HW : 6 * HW], False, True, HI)

            def hproj_pair(G, hTblk, s, t):
                """PE: h-projection for k-chunks {s, s+3} of step t from the
                transposed block hTblk [128,128] (cols 0:64 = chunk s,
                64:128 = chunk s+3)."""
                Gr, Gz = G[:, 0:HW], G[:, 512 : 512 + HW]
                Ghn = G[:, 1024 : 1024 + HW]
                for half, k in ((0, s), (1, s + NS)):
                    hk = hTblk[:, half * BL : (half + 1) * BL]
                    wb = whhTs[:, k * G3 : (k + 1) * G3]
                    last = k == KC - 1
                    mm(Gr[0:BL, :], hk, wb[:, 0:HW], False, last, LO)
                    mm(Gz[0:BL, :], hk, wb[:, HW : 2 * HW], False, last, LO)
                    mm(Ghn[0:BL, :], hk, wb[:, 2 * HW : 3 * HW], False, last, LO)
                    mm(Gr[BL:128, :], hk, wb[:, 3 * HW : 4 * HW], False, last, HI)
                    mm(Gz[BL:128, :], hk, wb[:, 4 * HW : 5 * HW], False, last, HI)
                    mm(Ghn[BL:128, :], hk, wb[:, 5 * HW : 6 * HW], False, last, HI)

            def tail_and_hproj(tt, Gp, NGp, h_pp, G_next, t_next):
                """ACT/DVE/GPS tail of step tt in NS slices; each finished
                slice is transposed and (if G_next) immediately feeds the
                next step's h-projection pair."""
                Gr, Gz = Gp[:, 0:HW], Gp[:, 512 : 512 + HW]
                Ghn = Gp[:, 1024 : 1024 + HW]
                Gin = NGp[:, 0:HW]

                r_s = work.tile([128, HW], f32, tag="r")
                z_s = work.tile([128, HW], f32, tag="z")
                tn = work.tile([128, HW], f32, tag="tn")
                tn2 = work.tile([128, HW], f32, tag="tn2")
                n_t = work.tile([128, HW], f32, tag="n")
                u = work.tile([128, HW], f32, tag="u")
                vm = work.tile([128, HW], f32, tag="vm")
                w1 = work.tile([128, HW], f32, tag="w1")
                h_new = state.tile([128, HW], f32, tag="h")
                hT = state.tile([128, HW], bf16, tag="hT")
                Tps = tpsum.tile([128, 512], f32, tag="T")

                sl = [slice(s * SW, (s + 1) * SW) for s in range(NS)]
                for s in range(NS):
                    c = sl[s]
                    nc.scalar.activation(r_s[:, c], Gr[:, c], AF.Sigmoid)
                    nc.scalar.activation(z_s[:, c], Gz[:, c], AF.Sigmoid)
                    nc.vector.tensor_mul(tn[:, c], r_s[:, c], Ghn[:, c])
                    nc.vector.tensor_add(tn2[:, c], tn[:, c], Gin[:, c])
                    nc.scalar.activation(n_t[:, c], tn2[:, c], AF.Tanh)
                    if tt > 0:
                        eng2.tensor_mul(u[:, c], z_s[:, c], h_pp[:, c])
                    eng2.tensor_scalar_sub(vm[:, c], z_s[:, c], 1.0)
                    eng2.tensor_mul(w1[:, c], vm[:, c], n_t[:, c])
                    if tt > 0:
                        eng2.tensor_sub(h_new[:, c], u[:, c], w1[:, c])
                    else:
                        eng2.tensor_scalar_mul(h_new[:, c], w1[:, c], -1.0)

                    # block transpose -> hT chunks {s, s+3}; bank-alternate
                    # the PSUM scratch so copy(s) and transpose(s+1) don't
                    # touch the same bank
                    tb = Tps[:, 128 : 256] if s % 2 else Tps[:, 0:128]
                    nc.tensor.transpose(tb, h_new[:, c], ident[:],
                                        tile_position=(0, 0))
                    nc.vector.tensor_copy(hT[:, c], tb)
                    if G_next is not None:
                        hproj_pair(G_next, hT[:, c], s, t_next)
                return h_new, hT

            def emit_fc_head(hT):
                fc1 = gpsum.tile([128, 1536], f32, tag="G")
                f1 = fc1[0:BL, 0:F1]
                mm(f1, ones[:], bfc1[:], True, False, (0, 0))
                for k in range(KC):
                    s, half = k % NS, k // NS
                    hk = hT[:, s * SW + half * BL : s * SW + (half + 1) * BL]
                    mm(f1, hk, wfc1Ts[:, k * F1 : (k + 1) * F1], False,
                       k == KC - 1, LO)
                o1 = work.tile([BL, F1], f32, tag="o1")
                nc.scalar.activation(o1[:], f1, AF.Relu)

                T2 = tpsum.tile([128, 512], f32, tag="T")
                nc.tensor.transpose(T2[:, 0:BL], o1[:, 0:128],
                                    ident[0:BL, 0:BL], tile_position=(0, 0))
                nc.tensor.transpose(T2[:, BL : 2 * BL], o1[:, 128:256],
                                    ident[0:BL, 0:BL], tile_position=(0, 0))
                o1T = work.tile([128, 2 * BL], bf16, tag="o1T")
                nc.vector.tensor_copy(o1T[:], T2[:, 0 : 2 * BL])

                fc2 = npsum.tile([128, 512], f32, tag="NG")
                f2 = fc2[0:BL, 0:C]
                mm(f2, ones[:], bfc2[:], True, False, (0, 0))
                mm(f2, o1T[:, 0:BL], wfc2Ts[:, 0:C], False, False, LO)
                mm(f2, o1T[:, BL : 2 * BL], wfc2Ts[:, C : 2 * C], False, True,
                   LO)
                lo = work.tile([BL, C], f32, tag="lo")
                nc.vector.tensor_copy(lo[:], f2)
                nc.sync.dma_start(out=out_d.ap(), in_=lo[:])

            def emit_body():
                NG = npsum.tile([128, 512], f32, tag="NG")
                G_prev = None
                h_prev = None  # h(tt-1) for the next tail call
                for t in range(T):
                    G = prologue(t)
                    if t == 0:
                        gin_part(0, NG)
                    else:
                        h_new, _ = tail_and_hproj(t - 1, G_prev, NG, h_prev,
                                                  G, t)
                        gin_part(t, NG)
                        h_prev = h_new
                    G_prev = G
                _, hT = tail_and_hproj(T - 1, G_prev, NG, h_prev, None, None)
                emit_fc_head(hT)

            if reps > 1:
                with tc.For_i(0, reps, 1):
                    emit_body()
            else:
                emit_body()

    nc.compile()
    return nc


def _prep_shared(w_ih, w_hh, b_ih, b_hh, w_fc1, b_fc1, w_fc2, b_fc2):
    import ml_dtypes

    bf = ml_dtypes.bfloat16
    f = np.float32

    def halves(wT):  # [K, 3H] -> [K, 6*HW]: r-lo z-lo n-lo r-hi z-hi n-hi
        out = np.empty((wT.shape[0], G3), np.float32)
        for g in range(3):  # r, z, n
            reg = wT[:, g * H : (g + 1) * H]
            out[:, g * HW : (g + 1) * HW] = reg[:, 0:HW]
            out[:, (3 + g) * HW : (4 + g) * HW] = reg[:, HW:H]
        return out

    whhT = np.ascontiguousarray(w_hh.T).astype(f)  # [H, 3H]
    whhTs = np.concatenate(
        [halves(whhT[k * 128 : (k + 1) * 128]) for k in range(KC)], axis=1
    )  # [128, KC*G3]
    wihTs = halves(np.ascontiguousarray(w_ih.T).astype(f))  # [128, G3]

    b_sum = (b_ih + b_hh).astype(f)  # for r, z regions
    bias2 = np.zeros((2, 4 * HW), np.float32)
    bias2[0, 0:HW] = b_sum[0:HW]  # r lo
    bias2[1, 0:HW] = b_sum[HW:H]  # r hi
    bias2[0, HW : 2 * HW] = b_sum[H : H + HW]  # z lo
    bias2[1, HW : 2 * HW] = b_sum[H + HW : 2 * H]  # z hi
    bias2[0, 2 * HW : 3 * HW] = b_ih[2 * H : 2 * H + HW]  # in lo
    bias2[1, 2 * HW : 3 * HW] = b_ih[2 * H + HW : 3 * H]  # in hi
    bias2[0, 3 * HW : 4 * HW] = b_hh[2 * H : 2 * H + HW]  # hn lo
    bias2[1, 3 * HW : 4 * HW] = b_hh[2 * H + HW : 3 * H]  # hn hi

    sel2 = np.zeros((2, 128), np.float32)
    sel2[0, 0:BL] = 1.0
    sel2[1, BL:128] = 1.0

    def kmajor(wT, kc, n):  # [kc*128, n] -> [128, kc*n]
        return np.ascontiguousarray(
            wT.reshape(kc, 128, n).transpose(1, 0, 2).reshape(128, kc * n)
        )

    shared = {
        "whhTs": whhTs.astype(bf),
        "wihTs": wihTs.astype(bf),
        "bias2": bias2.astype(bf),
        "sel2": sel2.astype(bf),
        "ones": np.ones((1, BL), bf),
        "wfc1Ts": kmajor(np.ascontiguousarray(w_fc1.T).astype(f), KC, F1).astype(bf),
        "bfc1": b_fc1.astype(f)[None, :].astype(bf),
        "wfc2Ts": kmajor(np.ascontiguousarray(w_fc2.T).astype(f), 2, C).astype(bf),
        "bfc2": b_fc2.astype(f)[None, :].astype(bf),
    }
    return shared


def _prep_in_maps(inputs):
    import ml_dtypes

    x = np.asarray(inputs["x"], dtype=np.float32)
    shared = _prep_shared(
        *(np.asarray(inputs[k], dtype=np.float32)
          for k in ("w_ih", "w_hh", "b_ih", "b_hh", "w_fc1", "b_fc1",
                    "w_fc2", "b_fc2"))
    )
    in_maps = []
    for c in range(NCORES):
        xs = x[c * BL : (c + 1) * BL]  # [64, T, I]
        xTc = np.ascontiguousarray(
            xs.transpose(2, 1, 0).reshape(128, T * BL)
        ).astype(ml_dtypes.bfloat16)
        in_maps.append({**shared, "xT": xTc})
    return in_maps


def _execute(in_maps, reps=1):
    from concourse.bass_utils import run_bass_kernel_spmd

    key = ("nc", reps)
    if key not in _CACHE:
        _CACHE[key] = _build_program(reps=reps)
    nc = _CACHE[key]
    res = run_bass_kernel_spmd(nc, in_maps, core_ids=list(range(NCORES)))
    out = np.concatenate([res.results[c]["logits"] for c in range(NCORES)], axis=0)
    return out.astype(np.float32), res


def kernel(**inputs):
    out, _ = _execute(_prep_in_maps(inputs))
    return out


# revision 8
# speedup vs baseline: 1.4655x; 1.0212x over previous
"""Trainium2 Bass kernel for the unrolled-GRU + FC-head problem (v2.2).

Math (per example b):
    gi[t] = x[t] @ w_ih.T + b_ih                       # [T, 3H]
    gh    = h  @ w_hh.T + b_hh                         # per step
    r = sig(gi_r + gh_r); z = sig(gi_z + gh_z)
    n = tanh(gi_n + r * gh_n)
    h = (1 - z) * n + z * h                            # T sequential steps
    out = relu(h @ w_fc1.T + b_fc1) @ w_fc2.T + b_fc2  # [C]

Sharding: data-parallel over batch. B=512 over 8 cores -> B_local=64.

Design (v1 f32r/[64,*] kernel measured 2.475 ms; v2 stacked 1.785 ms):
  - STACKED layout: every [64, 768]-logical tensor is stored [128, 384]:
    partitions 0:64 hold hidden cols 0:384 (lo), partitions 64:128 hold
    cols 384:768 (hi) for the same batch rows.  All ACT/DVE/GPS tail ops
    use all 128 lanes at half the free dim.  The hi half is produced by
    bf16 matmuls with tile_position=(0,64) (PE col groups 2,3); f32
    PSUM accumulation and an f32 tail keep rel err ~3e-3 (gate 2e-2).
  - Rank-2 bias matmuls (K=2 selector: row0 -> partitions 0:64, row1 ->
    64:128) write both halves of each bias region in one N=384 stream.
  - PSUM: G [128,1536] (Gr @0, Gz @512, Ghn @1024 - one bank each),
    double-buffered (6 banks) so step t+1's bias/x-proj stream while
    step t's tail runs (also keeps the PE HAM clock-gate warm);
    NG [128,512] holds Gin (single buffer; its bias/x are emitted after
    the tail reads of the previous Gin); T [128,512] transpose scratch.
  - 3-slice pipelined tail: h_new is produced in three 128-col stacked
    slices; each finished slice is block-transposed ([128,128] PE
    transpose = hT chunks {s, s+3} at once) and immediately feeds the
    next step's h-projection for those two k-chunks, overlapping the
    tail with PE streaming.
  - SBUF-only tail ops (u=z*h, vm=z-1, w1=vm*n, h=u-w1) run on GpSimd,
    PSUM-touching ones (tn, tn2) on DVE, sigmoids/tanh on ACT.
"""

import os
import sys

import numpy as np

if "/opt/trn_rl_repo" not in sys.path:
    sys.path.insert(0, "/opt/trn_rl_repo")

B, T, I, H, F1, C = 512, 128, 128, 768, 256, 10
NCORES = 8
BL = B // NCORES  # 64
G3 = 3 * H  # 2304
KC = H // 128  # 6 k-chunks of the hidden dim
HW = H // 2  # 384 stacked half-width
NS = 3  # tail slices
SW = HW // NS  # 128 slice width

GPS = os.environ.get("GRU_GPS", "1") == "1"

_CACHE = {}


def _build_program(reps=1):
    import concourse.bacc as bacc
    import concourse.mybir as mybir
    import concourse.tile as tile
    from concourse.masks import make_identity

    f32 = mybir.dt.float32
    bf16 = mybir.dt.bfloat16
    AF = mybir.ActivationFunctionType

    nc = bacc.Bacc(
        "TRN2",
        target_bir_lowering=False,
        debug=False,
        enable_asserts=False,
        num_devices=NCORES,
    )

    # ---- DRAM I/O ----
    xT_d = nc.dram_tensor("xT", [128, T * BL], bf16, kind="ExternalInput")
    whh_d = nc.dram_tensor("whhTs", [128, KC * G3], bf16, kind="ExternalInput")
    wih_d = nc.dram_tensor("wihTs", [128, G3], bf16, kind="ExternalInput")
    bias2_d = nc.dram_tensor("bias2", [2, 4 * HW], bf16, kind="ExternalInput")
    sel2_d = nc.dram_tensor("sel2", [2, 128], bf16, kind="ExternalInput")
    ones_d = nc.dram_tensor("ones", [1, BL], bf16, kind="ExternalInput")
    wfc1_d = nc.dram_tensor("wfc1Ts", [128, KC * F1], bf16, kind="ExternalInput")
    bfc1_d = nc.dram_tensor("bfc1", [1, F1], bf16, kind="ExternalInput")
    wfc2_d = nc.dram_tensor("wfc2Ts", [128, 2 * C], bf16, kind="ExternalInput")
    bfc2_d = nc.dram_tensor("bfc2", [1, C], bf16, kind="ExternalInput")
    out_d = nc.dram_tensor("logits", [BL, C], f32, kind="ExternalOutput")

    LO, HI = (0, 0), (0, 64)  # matmul tile_position for lo/hi output halves

    with tile.TileContext(nc) as tc:
        with (
            tc.tile_pool(name="const", bufs=1) as const,
            tc.tile_pool(name="state", bufs=2) as state,
            tc.tile_pool(name="work", bufs=2) as work,
            tc.tile_pool(name="gpsum", bufs=2, space="PSUM") as gpsum,
            tc.tile_pool(name="npsum", bufs=1, space="PSUM") as npsum,
            tc.tile_pool(name="tpsum", bufs=1, space="PSUM") as tpsum,
        ):
            def load(name, shape, dram, dt=bf16):
                t_ = const.tile(shape, dt, tag=name)
                nc.sync.dma_start(out=t_[:], in_=dram.ap())
                return t_

            xT = load("xT", [128, T * BL], xT_d)
            whhTs = load("whhTs", [128, KC * G3], whh_d)
            wihTs = load("wihTs", [128, G3], wih_d)
            bias2 = load("bias2", [2, 4 * HW], bias2_d)
            sel2 = load("sel2", [2, 128], sel2_d)
            ones = load("ones", [1, BL], ones_d)
            wfc1Ts = load("wfc1Ts", [128, KC * F1], wfc1_d)
            bfc1 = load("bfc1", [1, F1], bfc1_d)
            wfc2Ts = load("wfc2Ts", [128, 2 * C], wfc2_d)
            bfc2 = load("bfc2", [1, C], bfc2_d)

            ident = const.tile([128, 128], f32, tag="ident")
            make_identity(nc, ident[:])

            eng2 = nc.gpsimd if GPS else nc.vector

            def mm(out, lhsT, rhs, start, stop, tp):
                nc.tensor.matmul(out, lhsT, rhs, start=start, stop=stop,
                                 tile_position=tp)

            def prologue(t):
                """PE: G-buffer bias + x-proj (r, z) and Ghn bias.  G is
                double-buffered so these stream during step t-1's tail."""
                G = gpsum.tile([128, 1536], f32, tag="G")
                Gr, Gz = G[:, 0:HW], G[:, 512 : 512 + HW]
                Ghn = G[:, 1024 : 1024 + HW]
                xt = xT[:, t * BL : (t + 1) * BL]
                w = wihTs
                mm(Gr, sel2[:], bias2[:, 0:HW], True, False, (0, 0))
                mm(Gz, sel2[:], bias2[:, HW : 2 * HW], True, False, (0, 0))
                mm(Ghn, sel2[:], bias2[:, 3 * HW : 4 * HW], True, t == 0, (0, 0))
                mm(Gr[0:BL, :], xt, w[:, 0:HW], False, t == 0, LO)
                mm(Gz[0:BL, :], xt, w[:, HW : 2 * HW], False, t == 0, LO)
                mm(Gr[BL:128, :], xt, w[:, 3 * HW : 4 * HW], False, t == 0, HI)
                mm(Gz[BL:128, :], xt, w[:, 4 * HW : 5 * HW], False, t == 0, HI)
                return G

            def gin_part(t, NG):
                """PE: Gin bias + x-proj (single-buffered NG: emitted after
                the previous tail's Gin reads)."""
                Gin = NG[:, 0:HW]
                xt = xT[:, t * BL : (t + 1) * BL]
                w = wihTs
                mm(Gin, sel2[:], bias2[:, 2 * HW : 3 * HW], True, False, (0, 0))
                mm(Gin[0:BL, :], xt, w[:, 2 * HW : 3 * HW], False, True, LO)
                mm(Gin[BL:128, :], xt, w[:, 5 * HW : 6 * HW], False, True, HI)

            def hproj_pair(G, hTblk, s, t):
                """PE: h-projection for k-chunks {s, s+3} of step t from the
                transposed block hTblk [128,128] (cols 0:64 = chunk s,
                64:128 = chunk s+3).  For the final pair the four Gr matmuls
                are emitted first (despite the extra stationary reloads) so
                the tail's first sigmoid can start ~1.5us earlier."""
                Gr, Gz = G[:, 0:HW], G[:, 512 : 512 + HW]
                Ghn = G[:, 1024 : 1024 + HW]
                halves = ((0, s), (1, s + NS))

                def emit(region_first):
                    for half, k in halves:
                        hk = hTblk[:, half * BL : (half + 1) * BL]
                        wb = whhTs[:, k * G3 : (k + 1) * G3]
                        last = k == KC - 1
                        if region_first == "r":
                            mm(Gr[0:BL, :], hk, wb[:, 0:HW], False, last, LO)
                            mm(Gr[BL:128, :], hk, wb[:, 3 * HW : 4 * HW],
                               False, last, HI)
                        else:
                            mm(Gz[0:BL, :], hk, wb[:, HW : 2 * HW], False,
                               last, LO)
                            mm(Ghn[0:BL, :], hk, wb[:, 2 * HW : 3 * HW],
                               False, last, LO)
                            mm(Gz[BL:128, :], hk, wb[:, 4 * HW : 5 * HW],
                               False, last, HI)
                            mm(Ghn[BL:128, :], hk, wb[:, 5 * HW : 6 * HW],
                               False, last, HI)

                if s == NS - 1:
                    emit("r")
                    emit("zn")
                else:
                    for half, k in halves:
                        hk = hTblk[:, half * BL : (half + 1) * BL]
                        wb = whhTs[:, k * G3 : (k + 1) * G3]
                        last = k == KC - 1
                        mm(Gr[0:BL, :], hk, wb[:, 0:HW], False, last, LO)
                        mm(Gz[0:BL, :], hk, wb[:, HW : 2 * HW], False, last, LO)
                        mm(Ghn[0:BL, :], hk, wb[:, 2 * HW : 3 * HW], False,
                           last, LO)
                        mm(Gr[BL:128, :], hk, wb[:, 3 * HW : 4 * HW], False,
                           last, HI)
                        mm(Gz[BL:128, :], hk, wb[:, 4 * HW : 5 * HW], False,
                           last, HI)
                        mm(Ghn[BL:128, :], hk, wb[:, 5 * HW : 6 * HW], False,
                           last, HI)

            def tail_and_hproj(tt, Gp, NGp, h_pp, G_next, t_next):
                """ACT/DVE/GPS tail of step tt in NS slices; each finished
                slice is transposed and (if G_next) immediately feeds the
                next step's h-projection pair."""
                Gr, Gz = Gp[:, 0:HW], Gp[:, 512 : 512 + HW]
                Ghn = Gp[:, 1024 : 1024 + HW]
                Gin = NGp[:, 0:HW]

                r_s = work.tile([128, HW], f32, tag="r")
                z_s = work.tile([128, HW], f32, tag="z")
                tn = work.tile([128, HW], f32, tag="tn")
                tn2 = work.tile([128, HW], f32, tag="tn2")
                n_t = work.tile([128, HW], f32, tag="n")
                u = work.tile([128, HW], f32, tag="u")
                vm = work.tile([128, HW], f32, tag="vm")
                w1 = work.tile([128, HW], f32, tag="w1")
                h_new = state.tile([128, HW], f32, tag="h")
                hT = state.tile([128, HW], bf16, tag="hT")
                Tps = tpsum.tile([128, 512], f32, tag="T")

                sl = [slice(s * SW, (s + 1) * SW) for s in range(NS)]

                def sig_r(s):
                    nc.scalar.activation(r_s[:, sl[s]], Gr[:, sl[s]],
                                         AF.Sigmoid)

                def sig_z(s):
                    nc.scalar.activation(z_s[:, sl[s]], Gz[:, sl[s]],
                                         AF.Sigmoid)

                def tanh(s):
                    nc.scalar.activation(n_t[:, sl[s]], tn2[:, sl[s]], AF.Tanh)

                def tns(s):
                    c = sl[s]
                    nc.vector.tensor_mul(tn[:, c], r_s[:, c], Ghn[:, c])
                    nc.vector.tensor_add(tn2[:, c], tn[:, c], Gin[:, c])

                def hupd(s):
                    c = sl[s]
                    if tt > 0:
                        eng2.tensor_mul(u[:, c], z_s[:, c], h_pp[:, c])
                    eng2.tensor_scalar_sub(vm[:, c], z_s[:, c], 1.0)
                    eng2.tensor_mul(w1[:, c], vm[:, c], n_t[:, c])
                    if tt > 0:
                        eng2.tensor_sub(h_new[:, c], u[:, c], w1[:, c])
                    else:
                        eng2.tensor_scalar_mul(h_new[:, c], w1[:, c], -1.0)

                def tr_copy(s):
                    # block transpose -> hT chunks {s, s+3}; bank-alternate
                    # the PSUM scratch so copy(s) and transpose(s+1) don't
                    # touch the same bank
                    c = sl[s]
                    tb = Tps[:, 128:256] if s % 2 else Tps[:, 0:128]
                    nc.tensor.transpose(tb, h_new[:, c], ident[:],
                                        tile_position=(0, 0))
                    nc.vector.tensor_copy(hT[:, c], tb)

                def hproj(s):
                    if G_next is not None:
                        hproj_pair(G_next, hT[:, sl[s]], s, t_next)

                # Interleaved emission: ACT order keeps later slices'
                # sigmoids ahead of earlier tanh stalls; DVE order keeps the
                # next slice's muls ahead of transpose-gated hT copies.
                sig_r(0); sig_z(0); tns(0)
                sig_r(1); tanh(0); sig_z(1); tns(1)
                hupd(0); tr_copy(0); hproj(0)
                sig_r(2); tanh(1); sig_z(2); tns(2)
                hupd(1); tr_copy(1); hproj(1)
                tanh(2)
                hupd(2); tr_copy(2); hproj(2)
                return h_new, hT

            def emit_fc_head(hT):
                fc1 = gpsum.tile([128, 1536], f32, tag="G")
                f1 = fc1[0:BL, 0:F1]
                mm(f1, ones[:], bfc1[:], True, False, (0, 0))
                for k in range(KC):
                    s, half = k % NS, k // NS
                    hk = hT[:, s * SW + half * BL : s * SW + (half + 1) * BL]
                    mm(f1, hk, wfc1Ts[:, k * F1 : (k + 1) * F1], False,
                       k == KC - 1, LO)
                o1 = work.tile([BL, F1], f32, tag="o1")
                nc.scalar.activation(o1[:], f1, AF.Relu)

                T2 = tpsum.tile([128, 512], f32, tag="T")
                nc.tensor.transpose(T2[:, 0:BL], o1[:, 0:128],
                                    ident[0:BL, 0:BL], tile_position=(0, 0))
                nc.tensor.transpose(T2[:, BL : 2 * BL], o1[:, 128:256],
                                    ident[0:BL, 0:BL], tile_position=(0, 0))
                o1T = work.tile([128, 2 * BL], bf16, tag="o1T")
                nc.vector.tensor_copy(o1T[:], T2[:, 0 : 2 * BL])

                fc2 = npsum.tile([128, 512], f32, tag="NG")
                f2 = fc2[0:BL, 0:C]
                mm(f2, ones[:], bfc2[:], True, False, (0, 0))
                mm(f2, o1T[:, 0:BL], wfc2Ts[:, 0:C], False, False, LO)
                mm(f2, o1T[:, BL : 2 * BL], wfc2Ts[:, C : 2 * C], False, True,
                   LO)
                lo = work.tile([BL, C], f32, tag="lo")
                nc.vector.tensor_copy(lo[:], f2)
                nc.sync.dma_start(out=out_d.ap(), in_=lo[:])

            def emit_body():
                NG = npsum.tile([128, 512], f32, tag="NG")
                G_prev = None
                h_prev = None  # h(tt-1) for the next tail call
                for t in range(T):
                    G = prologue(t)
                    if t == 0:
                        gin_part(0, NG)
                    else:
                        h_new, _ = tail_and_hproj(t - 1, G_prev, NG, h_prev,
                                                  G, t)
                        gin_part(t, NG)
                        h_prev = h_new
                    G_prev = G
                _, hT = tail_and_hproj(T - 1, G_prev, NG, h_prev, None, None)
                emit_fc_head(hT)

            if reps > 1:
                with tc.For_i(0, reps, 1):
                    emit_body()
            else:
                emit_body()

    nc.compile()
    return nc


def _prep_shared(w_ih, w_hh, b_ih, b_hh, w_fc1, b_fc1, w_fc2, b_fc2):
    import ml_dtypes

    bf = ml_dtypes.bfloat16
    f = np.float32

    def halves(wT):  # [K, 3H] -> [K, 6*HW]: r-lo z-lo n-lo r-hi z-hi n-hi
        out = np.empty((wT.shape[0], G3), np.float32)
        for g in range(3):  # r, z, n
            reg = wT[:, g * H : (g + 1) * H]
            out[:, g * HW : (g + 1) * HW] = reg[:, 0:HW]
            out[:, (3 + g) * HW : (4 + g) * HW] = reg[:, HW:H]
        return out

    whhT = np.ascontiguousarray(w_hh.T).astype(f)  # [H, 3H]
    whhTs = np.concatenate(
        [halves(whhT[k * 128 : (k + 1) * 128]) for k in range(KC)], axis=1
    )  # [128, KC*G3]
    wihTs = halves(np.ascontiguousarray(w_ih.T).astype(f))  # [128, G3]

    b_sum = (b_ih + b_hh).astype(f)  # for r, z regions
    bias2 = np.zeros((2, 4 * HW), np.float32)
    bias2[0, 0:HW] = b_sum[0:HW]  # r lo
    bias2[1, 0:HW] = b_sum[HW:H]  # r hi
    bias2[0, HW : 2 * HW] = b_sum[H : H + HW]  # z lo
    bias2[1, HW : 2 * HW] = b_sum[H + HW : 2 * H]  # z hi
    bias2[0, 2 * HW : 3 * HW] = b_ih[2 * H : 2 * H + HW]  # in lo
    bias2[1, 2 * HW : 3 * HW] = b_ih[2 * H + HW : 3 * H]  # in hi
    bias2[0, 3 * HW : 4 * HW] = b_hh[2 * H : 2 * H + HW]  # hn lo
    bias2[1, 3 * HW : 4 * HW] = b_hh[2 * H + HW : 3 * H]  # hn hi

    sel2 = np.zeros((2, 128), np.float32)
    sel2[0, 0:BL] = 1.0
    sel2[1, BL:128] = 1.0

    def kmajor(wT, kc, n):  # [kc*128, n] -> [128, kc*n]
        return np.ascontiguousarray(
            wT.reshape(kc, 128, n).transpose(1, 0, 2).reshape(128, kc * n)
        )

    shared = {
        "whhTs": whhTs.astype(bf),
        "wihTs": wihTs.astype(bf),
        "bias2": bias2.astype(bf),
        "sel2": sel2.astype(bf),
        "ones": np.ones((1, BL), bf),
        "wfc1Ts": kmajor(np.ascontiguousarray(w_fc1.T).astype(f), KC, F1).astype(bf),
        "bfc1": b_fc1.astype(f)[None, :].astype(bf),
        "wfc2Ts": kmajor(np.ascontiguousarray(w_fc2.T).astype(f), 2, C).astype(bf),
        "bfc2": b_fc2.astype(f)[None, :].astype(bf),
    }
    return shared


def _prep_in_maps(inputs):
    import ml_dtypes

    x = np.asarray(inputs["x"], dtype=np.float32)
    shared = _prep_shared(
        *(np.asarray(inputs[k], dtype=np.float32)
          for k in ("w_ih", "w_hh", "b_ih", "b_hh", "w_fc1", "b_fc1",
                    "w_fc2", "b_fc2"))
    )
    in_maps = []
    for c in range(NCORES):
        xs = x[c * BL : (c + 1) * BL]  # [64, T, I]
        xTc = np.ascontiguousarray(
            xs.transpose(2, 1, 0).reshape(128, T * BL)
        ).astype(ml_dtypes.bfloat16)
        in_maps.append({**shared, "xT": xTc})
    return in_maps


def _execute(in_maps, reps=1):
    from concourse.bass_utils import run_bass_kernel_spmd

    key = ("nc", reps)
    if key not in _CACHE:
        _CACHE[key] = _build_program(reps=reps)
    nc = _CACHE[key]
    res = run_bass_kernel_spmd(nc, in_maps, core_ids=list(range(NCORES)))
    out = np.concatenate([res.results[c]["logits"] for c in range(NCORES)], axis=0)
    return out.astype(np.float32), res


def kernel(**inputs):
    out, _ = _execute(_prep_in_maps(inputs))
    return out
